# revision 7
# baseline (speedup 1.0000x reference)
"""Trainium2 Bass kernel for nn_Linear_regression (quadratic dot).

out0 = dot(w_lin, x) + dot(w_quad, x*x) + w[2W];  out1 = x[W//2] - out0

Shard all three streams along W across 8 cores. Stream x fp16, wq fp16,
wl raw float8_e3m4 consumed directly by DVE
mixed-dtype STT (no upcast pass, no extra dependency chain). 10.4MB/core
DMA per rep.

  - sync: DMA x(fp16) wl(e3m4) wq(fp16); per-tensor semaphores.
  - ACT: x2 = Square(x).
  - DVE: lin MAC via STT(wl8, 1.0, x) with per-tile accum column
    (1 elem/lane/cycle); V_q = wq*x2 packed TT (0.5).
  - PE: quad reduction via ones-matmuls into psum[1,512].
Exact rel err on the fixed inputs: ~1.4e-3 (gate 2e-2).
"""

import sys
from contextlib import ExitStack

for _p in ("/opt/trn_rl_repo", "/root/.axon_site/_ro/trn_rl_repo"):
    if _p not in sys.path:
        sys.path.append(_p)

import numpy as np
import ml_dtypes

W = 16777216
NCORES = 8
C = W // NCORES
P = 128
F = 8192
TILE = P * F
NT = C // TILE
NBUF = 2
MMF = 512

_cache = {}


def _pack(inputs: dict) -> list:
    x = np.asarray(inputs["x"], dtype=np.float32)
    w = np.asarray(inputs["weight"], dtype=np.float32)[0]
    xs = x.astype(np.float16).reshape(NCORES, NT * P, F)
    wls = w[:W].astype(ml_dtypes.float8_e3m4).reshape(NCORES, NT * P, F)
    wqs = w[W:2 * W].astype(np.float16).reshape(NCORES, NT * P, F)
    ones = np.ones((P, 1), dtype=np.float16)
    return [{"x": xs[c], "wl": wls[c], "wq": wqs[c], "ones": ones}
            for c in range(NCORES)]


def _build(reps: int = 1, nbuf: int = NBUF, x2buf: int = 2, vbuf: int = 2,
           f: int = F):
    import concourse.bass as bass
    from concourse import mybir

    f16 = mybir.dt.float16
    f32 = mybir.dt.float32
    f8 = mybir.dt.float8e3
    nc = bass.Bass()

    F = f
    NT = C // (P * F)
    NMM = F // MMF

    x_d = nc.declare_dram_parameter("x", [NT * P, F], f16, isOutput=False)
    wl_d = nc.declare_dram_parameter("wl", [NT * P, F], f8, isOutput=False)
    wq_d = nc.declare_dram_parameter("wq", [NT * P, F], f16, isOutput=False)
    ones_d = nc.declare_dram_parameter("ones", [P, 1], f16, isOutput=False)
    acc_d = nc.declare_dram_parameter("acc", [P, NT], f32, isOutput=True)
    qout_d = nc.declare_dram_parameter("qout", [1, MMF], f32, isOutput=True)

    mult = mybir.AluOpType.mult

    with ExitStack() as ctx:
        xb = [ctx.enter_context(nc.sbuf_tensor(f"xb{s}", [P, F], f16))
              for s in range(nbuf)]
        wl8b = [ctx.enter_context(nc.sbuf_tensor(f"wl8b{s}", [P, F], f8))
                for s in range(nbuf)]
        wqb = [ctx.enter_context(nc.sbuf_tensor(f"wqb{s}", [P, F], f16))
               for s in range(nbuf)]
        x2b = [ctx.enter_context(nc.sbuf_tensor(f"x2b{s}", [P, F], f16))
               for s in range(x2buf)]
        vqb = [ctx.enter_context(nc.sbuf_tensor(f"vqb{s}", [P, F], f16))
               for s in range(vbuf)]
        prodb = ctx.enter_context(nc.sbuf_tensor("prodb", [P, F], f16))
        onesb = ctx.enter_context(nc.sbuf_tensor("onesb", [P, 1], f16))
        accb = ctx.enter_context(nc.sbuf_tensor("accb", [P, NT], f32))
        drainb = ctx.enter_context(nc.sbuf_tensor("drainb", [1, MMF], f32))
        psq = ctx.enter_context(nc.psum_tensor("psq", [1, MMF], f32))

        sem_x = ctx.enter_context(nc.semaphore("sem_x"))
        sem_wl = ctx.enter_context(nc.semaphore("sem_wl"))
        sem_wq = ctx.enter_context(nc.semaphore("sem_wq"))
        sem_ones = ctx.enter_context(nc.semaphore("sem_ones"))
        sem_sq = ctx.enter_context(nc.semaphore("sem_sq"))
        sem_lp = ctx.enter_context(nc.semaphore("sem_lp"))
        sem_qp = ctx.enter_context(nc.semaphore("sem_qp"))
        sem_qg = ctx.enter_context(nc.semaphore("sem_qg"))
        sem_peq = ctx.enter_context(nc.semaphore("sem_peq"))
        sem_dr = ctx.enter_context(nc.semaphore("sem_dr"))
        sem_out = ctx.enter_context(nc.semaphore("sem_out"))

        with nc.Block() as block:

            G = NT * reps

            @block.sync
            def _(sync):
                sync.dma_start(onesb[:], ones_d[:]).then_inc(sem_ones, 16)
                for g in range(G):
                    i = g % NT
                    s = g % nbuf
                    rows = slice(i * P, (i + 1) * P)
                    if g >= nbuf:
                        gp = g - nbuf  # prior tile in slot s
                        sync.wait_ge(sem_lp, gp + 1)          # xb, wl8b
                        if gp % 2 == 0:
                            sync.wait_ge(sem_qp, gp // 2 + 1)  # wqb (DVE TT)
                        else:
                            sync.wait_ge(sem_qg, gp // 2 + 1)  # wqb (gps TT)
                    sync.dma_start(xb[s][:], x_d[rows, :]).then_inc(sem_x, 16)
                    sync.dma_start(wl8b[s][:], wl_d[rows, :]).then_inc(sem_wl, 16)
                    sync.dma_start(wqb[s][:], wq_d[rows, :]).then_inc(sem_wq, 16)
                sync.wait_ge(sem_lp, G)
                sync.wait_ge(sem_dr, 1)
                sync.dma_start(acc_d[:], accb[:]).then_inc(sem_out, 16)
                sync.dma_start(qout_d[:], drainb[:]).then_inc(sem_out, 16)
                sync.wait_ge(sem_out, 32)

            @block.scalar
            def _(scalar):
                for g in range(G):
                    s = g % nbuf
                    s2 = g % x2buf
                    scalar.wait_ge(sem_x, 16 * (g + 1))
                    if g >= x2buf:
                        gp = g - x2buf
                        if gp % 2 == 0:
                            scalar.wait_ge(sem_qp, gp // 2 + 1)
                        else:
                            scalar.wait_ge(sem_qg, gp // 2 + 1)
                    scalar.square(out=x2b[s2][:], in_=xb[s][:]).then_inc(sem_sq, 1)

            @block.vector
            def _(vector):
                for g in range(G):
                    i = g % NT
                    s = g % nbuf
                    s2 = g % x2buf
                    sv = g % vbuf
                    # HWDGE completes FIFO per queue; wl(g) lands after
                    # x(g), so one wait covers both inputs.
                    vector.wait_ge(sem_wl, 16 * (g + 1))
                    vector.scalar_tensor_tensor(
                        out=prodb[:], in0=wl8b[s][:], scalar=1.0, in1=xb[s][:],
                        op0=mult, op1=mult,
                        accum_out=accb[:, i:i + 1],
                    ).then_inc(sem_lp, 1)
                    if g % 2 == 0:
                        vector.wait_ge(sem_sq, g + 1)
                        vector.wait_ge(sem_wq, 16 * (g + 1))
                        if g >= vbuf:
                            vector.wait_ge(sem_peq, NMM * (g - vbuf + 1))
                        vector.tensor_tensor(
                            out=vqb[sv][:], in0=wqb[s][:], in1=x2b[s2][:], op=mult,
                        ).then_inc(sem_qp, 1)
                vector.wait_ge(sem_peq, NMM * G)
                vector.tensor_copy(out=drainb[:], in_=psq[:]).then_inc(sem_dr, 1)

            @block.gpsimd
            def _(gps):
                for g in range(G):
                    if g % 2 != 1:
                        continue
                    s = g % nbuf
                    s2 = g % x2buf
                    sv = g % vbuf
                    gps.wait_ge(sem_sq, g + 1)
                    gps.wait_ge(sem_wq, 16 * (g + 1))
                    if g >= vbuf:
                        gps.wait_ge(sem_peq, NMM * (g - vbuf + 1))
                    gps.tensor_tensor(
                        out=vqb[sv][:], in0=wqb[s][:], in1=x2b[s2][:], op=mult,
                    ).then_inc(sem_qg, 1)

            @block.tensor
            def _(tensor):
                tensor.wait_ge(sem_ones, 16)
                for g in range(G):
                    sv = g % vbuf
                    if g % 2 == 0:
                        tensor.wait_ge(sem_qp, g // 2 + 1)
                    else:
                        tensor.wait_ge(sem_qg, g // 2 + 1)
                    for c in range(NMM):
                        tensor.matmul(
                            psq[:, :], onesb[:, :],
                            vqb[sv][:, c * MMF:(c + 1) * MMF],
                            start=(g == 0 and c == 0), stop=(g == G - 1 and c == NMM - 1),
                            skip_group_check=True,
                        ).then_inc(sem_peq, 1)

    return nc


def _run(inputs: dict, trace: bool = False, tmpdir: str | None = None):
    from concourse.bass_utils import run_bass_kernel_spmd

    if "nc" not in _cache:
        _cache["nc"] = _build(reps=1)
    nc = _cache["nc"]

    x = np.asarray(inputs["x"], dtype=np.float32)
    w = np.asarray(inputs["weight"], dtype=np.float32)[0]

    in_maps = _pack(inputs)
    res = run_bass_kernel_spmd(
        nc, in_maps, core_ids=list(range(NCORES)),
        trace=trace, tmpdir=tmpdir,
    )

    total = np.float64(0.0)
    for c in range(NCORES):
        total += res.results[c]["acc"].astype(np.float64).sum()
        total += res.results[c]["qout"].astype(np.float64).sum()

    out0 = np.float32(total + np.float64(w[2 * W]))
    out1 = np.float32(x[W // 2]) - out0
    return np.stack([out0, out1]).astype(np.float32), res


def kernel(**inputs) -> np.ndarray:
    out, _ = _run(inputs)
    return out
